# revision 11
# baseline (speedup 1.0000x reference)
"""Trainium2 Bass kernel for the DendriticLayer LIF problem.

Math (reference):
    mask[r, c] = (r % 4) == (c // 1024)            # block-diagonal per branch
    dense      = (x @ (W*mask).T + b).reshape(B, OUT, 4)
    d_new      = beta * d_input + (1-beta) * dense
    l_input    = d_new.sum(-1)
    mem_new    = alpha*mem + (1-alpha)*l_input - spike
    spike_new  = (mem_new - 1 > 0)

Because the mask is block-diagonal, row o*4+j of W only touches input block j.
Folding the per-row scales (1-alpha[o])*(1-beta[o,j]) into those blocks and
concatenating the 4 blocks along the contraction axis turns everything into a
single dense matmul:

    V[j*1024+k, o] = (1-alpha[o]) * (1-beta[o,j]) * W[o*4+j, j*1024+k]
    c2[o]          = (1-alpha[o]) * sum_j (1-beta[o,j]) * b[o*4+j]
    mem_new        = alpha*mem - spike + c2 + x @ V          (+ beta*d_input
                                                              term, host-side,
                                                              zero by spec)

On device (per core): 8 cores = 2 batch halves x 4 output quarters, so each
core computes a [512 out, 512 batch] block of (x @ V).T:

    psum[o, b] = sum_{k-pairs} Vq[2k, o]*x[2k, b] + Vq[2k+1, o]*x[2k+1, b]

using fp8e4m3 DoubleRow matmuls (2 fp8 weights per PE cell -> 256-deep
contraction per instruction; measured 216 ns per N=512 MM, i.e. 2x bf16
throughput).  x is 0/1 so it is exact in fp8; V is scaled by 2^11 so the
psum sits in e4m3 range and the PSUM->SBUF evacuation is a plain fp8 cast
(output = 256 KB).  Measured end-to-end rel err ~6e-4 vs the 2e-2 gate.

DMA: the host packs x|V chunk-major so every chunk is one fully contiguous
HBM span (best DMA efficiency), and chunks alternate between the two HWDGE
rings (Scalar engine's qActDynamicHW and Sync's qSPDynamicHW) so triggers
and transfers overlap.  Scalar goes first because its preamble retires
~1.3us before Sync's.  A few warm-up matmuls on an (uninitialized) tile
keep the PE busy from the moment the framework barrier drops so HAM
un-throttles the clock as early as possible.  The last chunk's matmuls are
grouped per PSUM bank so each bank's evacuation + store DMA overlaps the
remaining banks' matmuls.  The LIF elementwise update is a cheap host-side
epilogue.
"""

import os
import sys

import numpy as np
import ml_dtypes

for _p in ("/opt/trn_rl_repo",):
    if os.path.isdir(_p) and _p not in sys.path:
        sys.path.append(_p)

import concourse.bass as bass  # noqa: E402
import concourse.tile as tile  # noqa: E402
from concourse import bacc, mybir  # noqa: E402
from concourse._compat import with_exitstack  # noqa: E402
from concourse import bass_utils  # noqa: E402

# Problem shapes (hardcoded per harness contract)
B, IN, OUT, NB = 1024, 4096, 2048, 4
NCORES = 8
BH = B // 2                # 512 batch rows per core (2 halves)
OQ = OUT // 4              # 512 output rows per core (4 quarters)
P = 128                    # partition dim
KT = IN // P               # 32 contraction tiles of 128
KP = KT // 2               # 16 DoubleRow k-pairs
OTILES = OQ // P           # 4 output partition tiles
CW = BH + OQ               # stream columns per k-tile (x | v)
VTH = 1.0
NWARM = 26                 # dummy warm-up matmuls (N=128)
SC = float(2 ** 11)        # V scale: psum absmax ~164 < e4m3 max 240

# k-tiles per DMA chunk (even so DoubleRow pairs stay inside a chunk) and
# the HWDGE ring each loads on (True = sync, False = scalar).  Sync first
# and last: the scalar ring starts ~1us slower, so it gets fewer bytes and
# never the critical last chunk.  The last chunk is small so the
# post-stream compute tail is short.
CHUNKS = [2, 4, 4, 6, 6, 4, 4, 2]
RINGS = [1, 0, 1, 0, 1, 0, 1, 1]
assert sum(CHUNKS) == KT and all(c % 2 == 0 for c in CHUNKS)

BF16 = mybir.dt.bfloat16
FP8 = mybir.dt.float8e4
F32 = mybir.dt.float32
BF16_NP = ml_dtypes.bfloat16
FP8_NP = ml_dtypes.float8_e4m3
DR = mybir.MatmulPerfMode.DoubleRow


@with_exitstack
def _body(ctx, tc, outt, sv):
    nc = tc.nc

    svpool = ctx.enter_context(tc.tile_pool(name="svpool", bufs=1))
    opool = ctx.enter_context(tc.tile_pool(name="opool", bufs=1))
    wpool = ctx.enter_context(tc.tile_pool(name="wpool", bufs=1))
    ppool = ctx.enter_context(tc.tile_pool(name="ppool", bufs=1, space="PSUM"))

    # PE warm-up: matmuls on a zeroed tile (psum never read), dependent only
    # on a VectorE memset, so they start right after the framework preamble
    # barrier drops -- HAM's 3.4us activity window starts counting as early
    # as possible.
    zt = wpool.tile([P, P], BF16, name="zt")
    nc.vector.memset(zt[:], 0.0)
    ps_warm = ppool.tile([P, P], F32, name="ps_warm")
    for w in range(NWARM):
        nc.tensor.matmul(ps_warm[:], zt[:], zt[:], start=True, stop=True,
                         skip_group_check=True)

    # Streaming loads: each chunk is one fully contiguous HBM span
    # (host packs chunk-major), alternating scalar/sync HWDGE rings.
    sg, kstart = [], []
    k0 = 0
    for g, ck in enumerate(CHUNKS):
        kstart.append(k0)
        t_ = svpool.tile([P, ck, CW], FP8, name=f"sg{g}")
        src = sv[k0 * P * CW:(k0 + ck) * P * CW].rearrange(
            "(p k c) -> p k c", p=P, c=CW)
        eng = nc.sync if RINGS[g] else nc.scalar
        eng.dma_start(t_[:], src)
        sg.append(t_)
        k0 += ck

    ps = [ppool.tile([P, BH], F32, name=f"ps{t}") for t in range(OTILES)]
    # One SBUF staging tile per bank: shared tiles create false cross-engine
    # dependencies in the Tile scheduler (copy of bank 3 waiting on bank 1's
    # store DMA).
    out_s = [opool.tile([P, BH], FP8, name=f"out_s{t}") for t in range(OTILES)]
    outt_r = outt.rearrange("(t p) b -> t p b", p=P)

    # Dense accumulation: 16 DoubleRow k-pairs x 4 o-tiles.  The last chunk
    # runs bank-major so each bank finishes (and drains) while later banks
    # still compute.
    last = len(CHUNKS) - 1
    for g, ck in enumerate(CHUNKS):
        pairs = [(kk, (kstart[g] + kk) // 2) for kk in range(0, ck, 2)]
        order = ([(t, kk, kp) for t in range(OTILES) for kk, kp in pairs]
                 if g == last else
                 [(t, kk, kp) for kk, kp in pairs for t in range(OTILES)])
        for t, kk, kp in order:
            x_ap = sg[g][:, kk:kk + 2, 0:BH]
            v_ap = sg[g][:, kk:kk + 2, BH + t * P:BH + (t + 1) * P]
            nc.tensor.matmul(
                ps[t][:], v_ap, x_ap,
                start=(kp == 0), stop=(kp == KP - 1),
                perf_mode=DR,
            )
            if g == last and kp == KP - 1:
                # Evacuate this bank (f32 psum -> fp8 cast) and store it,
                # overlapping the remaining banks' matmuls.  ScalarE takes
                # banks 0/2, VectorE banks 1/3, so the two engines drain in
                # parallel with no shared-tile false dependencies.
                if t % 2 == 0:
                    nc.scalar.copy(out_s[t][:], ps[t][:])
                else:
                    nc.vector.tensor_copy(out_s[t][:], ps[t][:])
                eng = nc.sync if t % 2 == 0 else nc.scalar
                eng.dma_start(outt_r[t], out_s[t][:])


_CACHE = {}


def build():
    if "nc" in _CACHE:
        return _CACHE["nc"]
    nc = bacc.Bacc(
        "TRN2",
        target_bir_lowering=False,
        debug=False,
        enable_asserts=False,
        num_devices=NCORES,
    )
    sv = nc.dram_tensor("sv", [KT * P * CW], FP8, kind="ExternalInput").ap()
    outt = nc.dram_tensor("outt", [OTILES * P, BH], FP8,
                          kind="ExternalOutput").ap()
    with tile.TileContext(nc) as tc:
        _body(tc, outt, sv)
    nc.compile()
    _CACHE["nc"] = nc
    return nc


def _sigmoid64(x):
    return 1.0 / (1.0 + np.exp(-x.astype(np.float64)))


def prep_host(inputs):
    """Fold scales into weights; build per-core input maps."""
    W = np.asarray(inputs["W"])
    b = np.asarray(inputs["b"])
    alpha = _sigmoid64(np.asarray(inputs["tau_m"]))        # [OUT]
    beta = _sigmoid64(np.asarray(inputs["tau_n"]))         # [OUT, NB]
    S = IN // NB

    W4 = W.reshape(OUT, NB, IN)                            # row o*4+j = W4[o, j]
    s = (1.0 - alpha)[:, None] * (1.0 - beta)              # [OUT, NB] f64
    blocks = [
        (W4[:, j, j * S:(j + 1) * S].astype(np.float64) * s[:, j:j + 1]).T
        for j in range(NB)
    ]
    V = np.concatenate(blocks, axis=0)                     # [IN, OUT] f64
    c2 = ((1.0 - alpha) * np.sum((1.0 - beta) * b.reshape(OUT, NB).astype(np.float64), axis=1))

    # X packed partition-major per k-tile, split into batch halves:
    # xk[h][k, p, b] = X[h*BH + b, k*128+p], exact in fp8 (x is 0/1)
    Xt = np.asarray(inputs["input_spike"]).T               # [IN, B]
    xk = [
        Xt[:, h * BH:(h + 1) * BH].astype(FP8_NP).reshape(KT, P, BH)
        for h in range(2)
    ]
    # V quarters, scaled into e4m3 range
    Vs = np.clip(V * SC, -240.0, 240.0).astype(FP8_NP)     # [IN, OUT]

    in_maps = []
    for c in range(NCORES):
        h, q = c % 2, c // 2
        vkq = Vs[:, q * OQ:(q + 1) * OQ].reshape(KT, P, OQ)
        stream = np.concatenate([xk[h], vkq], axis=2)      # [KT, P, CW]
        # chunk-major: each chunk g is a contiguous [P, ck*CW] block
        parts, k0 = [], 0
        for ck in CHUNKS:
            parts.append(np.ascontiguousarray(
                stream[k0:k0 + ck].transpose(1, 0, 2)).reshape(-1))
            k0 += ck
        SV = np.concatenate(parts)
        in_maps.append({"sv": SV})
    return in_maps, alpha, beta, c2


def finish_host(shards, inputs, alpha, beta, c2):
    l_part = np.empty((B, OUT), dtype=np.float32)
    inv_sc = np.float32(1.0 / SC)
    for c in range(NCORES):
        h, q = c % 2, c // 2
        arr = shards[c].astype(np.float32)                 # [4*P, BH]
        l_part[h * BH:(h + 1) * BH, q * OQ:(q + 1) * OQ] = arr.T * inv_sc
    a32 = alpha.astype(np.float32)[None, :]
    c32 = c2.astype(np.float32)[None, :]
    mem = np.asarray(inputs["mem"])
    spk = np.asarray(inputs["spike"])
    mem_new = mem * a32 - spk + c32 + l_part               # fp32 elementwise
    d_input = np.asarray(inputs["d_input"])
    if d_input.any():
        corr = (
            np.einsum("boj,oj->bo", d_input.astype(np.float64), beta)
            * (1.0 - alpha)[None, :]
        ).astype(np.float32)
        mem_new = mem_new + corr
    spike_new = ((mem_new - np.float32(VTH)) > 0).astype(np.float32)
    return mem_new, spike_new


def _axon_reset():
    """Recover wedged NeuronCores (NRT_EXEC_UNIT_UNRECOVERABLE) via the
    axon client's reset entry point."""
    try:
        import ctypes
        import jax
        jax.devices()
        lib = ctypes.CDLL("/opt/axon/libaxon_pjrt.so")
        lib.axon_reset.restype = ctypes.c_int64
        lib.axon_reset()
    except Exception:
        pass


def run(inputs, trace=False):
    nc = build()
    in_maps, alpha, beta, c2 = prep_host(inputs)
    kwargs = {}
    if trace:
        bass_utils.upload_artifacts = lambda tmpdir: tmpdir
        _ensure_ntff_hook()
        kwargs["trace"] = True
    try:
        res = bass_utils.run_bass_kernel_spmd(
            nc, in_maps, core_ids=list(range(NCORES)), **kwargs
        )
    except Exception:
        _axon_reset()
        res = bass_utils.run_bass_kernel_spmd(
            nc, in_maps, core_ids=list(range(NCORES)), **kwargs
        )
    shards = [res.results[c]["outt"] for c in range(NCORES)]
    mem_new, spike_new = finish_host(shards, inputs, alpha, beta, c2)
    return (mem_new, spike_new), res


def _ensure_ntff_hook():
    try:
        from antenv.axon_hooks import get_axon_ntff_profile_hook  # noqa: F401
        return
    except ImportError:
        pass
    import types
    try:
        import trn_agent_boot.trn_boot as tb
        hook = tb._ntff_profile_via_ctypes("/opt/axon/libaxon_pjrt.so")
    except Exception:
        hook = None
    mod = types.ModuleType("antenv.axon_hooks")
    mod.get_axon_ntff_profile_hook = lambda: hook
    mod.set_axon_ntff_profile_hook = lambda h: None
    import antenv
    sys.modules["antenv.axon_hooks"] = mod
    antenv.axon_hooks = mod


def kernel(**inputs):
    (mem_new, spike_new), _ = run(inputs, trace=False)
    return mem_new, spike_new


# revision 16
# speedup vs baseline: 1.0434x; 1.0434x over previous
"""Trainium2 Bass kernel for the DendriticLayer LIF problem.

Math (reference):
    mask[r, c] = (r % 4) == (c // 1024)            # block-diagonal per branch
    dense      = (x @ (W*mask).T + b).reshape(B, OUT, 4)
    d_new      = beta * d_input + (1-beta) * dense
    l_input    = d_new.sum(-1)
    mem_new    = alpha*mem + (1-alpha)*l_input - spike
    spike_new  = (mem_new - 1 > 0)

Because the mask is block-diagonal, row o*4+j of W only touches input block j.
Folding the per-row scales (1-alpha[o])*(1-beta[o,j]) into those blocks and
concatenating the 4 blocks along the contraction axis turns everything into a
single dense matmul:

    V[j*1024+k, o] = (1-alpha[o]) * (1-beta[o,j]) * W[o*4+j, j*1024+k]
    c2[o]          = (1-alpha[o]) * sum_j (1-beta[o,j]) * b[o*4+j]
    mem_new        = alpha*mem - spike + c2 + x @ V          (+ beta*d_input
                                                              term, host-side,
                                                              zero by spec)

On device (per core): 8 cores = 2 batch halves x 4 output quarters, so each
core computes a [512 out, 512 batch] block of (x @ V).T:

    psum[o, b] = sum_{k-pairs} Vq[2k, o]*x[2k, b] + Vq[2k+1, o]*x[2k+1, b]

using fp8e4m3 DoubleRow matmuls (2 fp8 weights per PE cell -> 256-deep
contraction per instruction; measured 216 ns per N=512 MM, i.e. 2x bf16
throughput).  x is 0/1 so it is exact in fp8; V is scaled by 2^11 so the
psum sits in e4m3 range and the PSUM->SBUF evacuation is a plain fp8 cast
(output = 256 KB).  Measured end-to-end rel err ~6e-4 vs the 2e-2 gate.

DMA: the host packs x|V chunk-major so every chunk is one fully contiguous
HBM span (best DMA efficiency), and chunks alternate between the two HWDGE
rings (Scalar engine's qActDynamicHW and Sync's qSPDynamicHW) so triggers
and transfers overlap.  Scalar goes first because its preamble retires
~1.3us before Sync's.  A few warm-up matmuls on an (uninitialized) tile
keep the PE busy from the moment the framework barrier drops so HAM
un-throttles the clock as early as possible.  The last chunk's matmuls are
grouped per PSUM bank so each bank's evacuation + store DMA overlaps the
remaining banks' matmuls.  The LIF elementwise update is a cheap host-side
epilogue.
"""

import os
import sys

import numpy as np
import ml_dtypes

for _p in ("/opt/trn_rl_repo",):
    if os.path.isdir(_p) and _p not in sys.path:
        sys.path.append(_p)

import concourse.bass as bass  # noqa: E402
import concourse.tile as tile  # noqa: E402
from concourse import bacc, mybir  # noqa: E402
from concourse._compat import with_exitstack  # noqa: E402
from concourse import bass_utils  # noqa: E402

# Problem shapes (hardcoded per harness contract)
B, IN, OUT, NB = 1024, 4096, 2048, 4
NCORES = 8
BH = B // 2                # 512 batch rows per core (2 halves)
OQ = OUT // 4              # 512 output rows per core (4 quarters)
P = 128                    # partition dim
KT = IN // P               # 32 contraction tiles of 128
KP = KT // 2               # 16 DoubleRow k-pairs
OTILES = OQ // P           # 4 output partition tiles
CW = BH + OQ               # stream columns per k-tile (x | v)
VTH = 1.0
NWARM = 34                 # warm-up matmuls: bridge preamble -> first data
SC = float(2 ** 11)        # V scale: psum absmax ~164 < e4m3 max 240

# k-tiles per DMA chunk (even so DoubleRow pairs stay inside a chunk) and
# the HWDGE ring each loads on (True = sync, False = scalar).  Sync first
# and last: the scalar ring starts ~1us slower.  Small chunks early for
# fine-grained PE chasing, small last chunk for a short compute tail.
CHUNKS = [2, 2, 4, 4, 6, 6, 4, 4]
RINGS = [1, 0, 1, 0, 1, 0, 0, 1]
assert sum(CHUNKS) == KT and all(c % 2 == 0 for c in CHUNKS)

BF16 = mybir.dt.bfloat16
FP8 = mybir.dt.float8e4
F32 = mybir.dt.float32
BF16_NP = ml_dtypes.bfloat16
FP8_NP = ml_dtypes.float8_e4m3
DR = mybir.MatmulPerfMode.DoubleRow


@with_exitstack
def _body(ctx, tc, outt, sv):
    nc = tc.nc

    svpool = ctx.enter_context(tc.tile_pool(name="svpool", bufs=1))
    opool = ctx.enter_context(tc.tile_pool(name="opool", bufs=1))
    wpool = ctx.enter_context(tc.tile_pool(name="wpool", bufs=1))
    ppool = ctx.enter_context(tc.tile_pool(name="ppool", bufs=1, space="PSUM"))

    # PE warm-up: matmuls on a zeroed tile (psum never read), dependent only
    # on a VectorE memset, so they start right after the framework preamble
    # barrier drops -- HAM's 3.4us activity window starts counting as early
    # as possible.
    zt = wpool.tile([P, P], BF16, name="zt")
    nc.vector.memset(zt[:], 0.0)
    ps_warm = ppool.tile([P, P], F32, name="ps_warm")
    for w in range(NWARM):
        nc.tensor.matmul(ps_warm[:], zt[:], zt[:], start=True, stop=True,
                         skip_group_check=True)

    # Streaming loads: each chunk is one fully contiguous HBM span
    # (host packs chunk-major), alternating scalar/sync HWDGE rings.
    sg, kstart = [], []
    k0 = 0
    for g, ck in enumerate(CHUNKS):
        kstart.append(k0)
        t_ = svpool.tile([P, ck, CW], FP8, name=f"sg{g}")
        src = sv[k0 * P * CW:(k0 + ck) * P * CW].rearrange(
            "(p k c) -> p k c", p=P, c=CW)
        eng = nc.sync if RINGS[g] else nc.scalar
        eng.dma_start(t_[:], src)
        sg.append(t_)
        k0 += ck

    ps = [ppool.tile([P, BH], F32, name=f"ps{t}") for t in range(OTILES)]
    # One shared staging tile so the store DMAs cover multiple banks with
    # wider (1.5KB) per-partition descriptors; its writers are the 4 copies
    # only, so no false cross-engine deps arise.
    out_s = opool.tile([P, OTILES * BH], FP8, name="out_s")

    # Dense accumulation: 16 DoubleRow k-pairs x 4 o-tiles.  The last chunk
    # runs bank-major so each bank finishes (and drains) while later banks
    # still compute.
    last = len(CHUNKS) - 1
    for g, ck in enumerate(CHUNKS):
        pairs = [(kk, (kstart[g] + kk) // 2) for kk in range(0, ck, 2)]
        order = ([(t, kk, kp) for t in range(OTILES) for kk, kp in pairs]
                 if g == last else
                 [(t, kk, kp) for kk, kp in pairs for t in range(OTILES)])
        for t, kk, kp in order:
            x_ap = sg[g][:, kk:kk + 2, 0:BH]
            v_ap = sg[g][:, kk:kk + 2, BH + t * P:BH + (t + 1) * P]
            nc.tensor.matmul(
                ps[t][:], v_ap, x_ap,
                start=(kp == 0), stop=(kp == KP - 1),
                perf_mode=DR,
            )
            if g == last and kp == KP - 1:
                # Evacuate this bank (f32 psum -> fp8 cast), overlapping the
                # remaining banks' matmuls.  ScalarE takes banks 0/2, VectorE
                # banks 1/3 so the engines drain in parallel.  Banks 0-2 go
                # out in one wide DMA as soon as they are down; bank 3 (the
                # critical path) goes alone on the other ring.
                sl = slice(t * BH, (t + 1) * BH)
                if t % 2 == 0:
                    nc.scalar.copy(out_s[:, sl], ps[t][:])
                else:
                    nc.vector.tensor_copy(out_s[:, sl], ps[t][:])
                if t == 2:
                    nc.sync.dma_start(outt[:, 0:3 * BH], out_s[:, 0:3 * BH])
                elif t == 3:
                    nc.scalar.dma_start(outt[:, 3 * BH:], out_s[:, sl])


_CACHE = {}


def build():
    if "nc" in _CACHE:
        return _CACHE["nc"]
    nc = bacc.Bacc(
        "TRN2",
        target_bir_lowering=False,
        debug=False,
        enable_asserts=False,
        num_devices=NCORES,
    )
    sv = nc.dram_tensor("sv", [KT * P * CW], FP8, kind="ExternalInput").ap()
    outt = nc.dram_tensor("outt", [P, OTILES * BH], FP8,
                          kind="ExternalOutput").ap()
    with tile.TileContext(nc) as tc:
        _body(tc, outt, sv)
    nc.compile()
    _CACHE["nc"] = nc
    return nc


def _sigmoid64(x):
    return 1.0 / (1.0 + np.exp(-x.astype(np.float64)))


def prep_host(inputs):
    """Fold scales into weights; build per-core input maps."""
    W = np.asarray(inputs["W"])
    b = np.asarray(inputs["b"])
    alpha = _sigmoid64(np.asarray(inputs["tau_m"]))        # [OUT]
    beta = _sigmoid64(np.asarray(inputs["tau_n"]))         # [OUT, NB]
    S = IN // NB

    W4 = W.reshape(OUT, NB, IN)                            # row o*4+j = W4[o, j]
    s = (1.0 - alpha)[:, None] * (1.0 - beta)              # [OUT, NB] f64
    blocks = [
        (W4[:, j, j * S:(j + 1) * S].astype(np.float64) * s[:, j:j + 1]).T
        for j in range(NB)
    ]
    V = np.concatenate(blocks, axis=0)                     # [IN, OUT] f64
    c2 = ((1.0 - alpha) * np.sum((1.0 - beta) * b.reshape(OUT, NB).astype(np.float64), axis=1))

    # X packed partition-major per k-tile, split into batch halves:
    # xk[h][k, p, b] = X[h*BH + b, k*128+p], exact in fp8 (x is 0/1)
    Xt = np.asarray(inputs["input_spike"]).T               # [IN, B]
    xk = [
        Xt[:, h * BH:(h + 1) * BH].astype(FP8_NP).reshape(KT, P, BH)
        for h in range(2)
    ]
    # V quarters, scaled into e4m3 range
    Vs = np.clip(V * SC, -240.0, 240.0).astype(FP8_NP)     # [IN, OUT]

    in_maps = []
    for c in range(NCORES):
        h, q = c % 2, c // 2
        vkq = Vs[:, q * OQ:(q + 1) * OQ].reshape(KT, P, OQ)
        stream = np.concatenate([xk[h], vkq], axis=2)      # [KT, P, CW]
        # chunk-major: each chunk g is a contiguous [P, ck*CW] block
        parts, k0 = [], 0
        for ck in CHUNKS:
            parts.append(np.ascontiguousarray(
                stream[k0:k0 + ck].transpose(1, 0, 2)).reshape(-1))
            k0 += ck
        SV = np.concatenate(parts)
        in_maps.append({"sv": SV})
    return in_maps, alpha, beta, c2


def finish_host(shards, inputs, alpha, beta, c2):
    l_part = np.empty((B, OUT), dtype=np.float32)
    inv_sc = np.float32(1.0 / SC)
    for c in range(NCORES):
        h, q = c % 2, c // 2
        arr = shards[c].astype(np.float32)                 # [P, OTILES*BH]
        Oc = arr.reshape(P, OTILES, BH).transpose(1, 0, 2).reshape(OQ, BH)
        l_part[h * BH:(h + 1) * BH, q * OQ:(q + 1) * OQ] = Oc.T * inv_sc
    a32 = alpha.astype(np.float32)[None, :]
    c32 = c2.astype(np.float32)[None, :]
    mem = np.asarray(inputs["mem"])
    spk = np.asarray(inputs["spike"])
    mem_new = mem * a32 - spk + c32 + l_part               # fp32 elementwise
    d_input = np.asarray(inputs["d_input"])
    if d_input.any():
        corr = (
            np.einsum("boj,oj->bo", d_input.astype(np.float64), beta)
            * (1.0 - alpha)[None, :]
        ).astype(np.float32)
        mem_new = mem_new + corr
    spike_new = ((mem_new - np.float32(VTH)) > 0).astype(np.float32)
    return mem_new, spike_new


def _axon_reset():
    """Recover wedged NeuronCores (NRT_EXEC_UNIT_UNRECOVERABLE) via the
    axon client's reset entry point."""
    try:
        import ctypes
        import jax
        jax.devices()
        lib = ctypes.CDLL("/opt/axon/libaxon_pjrt.so")
        lib.axon_reset.restype = ctypes.c_int64
        lib.axon_reset()
    except Exception:
        pass


def run(inputs, trace=False):
    nc = build()
    in_maps, alpha, beta, c2 = prep_host(inputs)
    kwargs = {}
    if trace:
        bass_utils.upload_artifacts = lambda tmpdir: tmpdir
        _ensure_ntff_hook()
        kwargs["trace"] = True
    try:
        res = bass_utils.run_bass_kernel_spmd(
            nc, in_maps, core_ids=list(range(NCORES)), **kwargs
        )
    except Exception:
        _axon_reset()
        res = bass_utils.run_bass_kernel_spmd(
            nc, in_maps, core_ids=list(range(NCORES)), **kwargs
        )
    shards = [res.results[c]["outt"] for c in range(NCORES)]
    mem_new, spike_new = finish_host(shards, inputs, alpha, beta, c2)
    return (mem_new, spike_new), res


def _ensure_ntff_hook():
    try:
        from antenv.axon_hooks import get_axon_ntff_profile_hook  # noqa: F401
        return
    except ImportError:
        pass
    import types
    try:
        import trn_agent_boot.trn_boot as tb
        hook = tb._ntff_profile_via_ctypes("/opt/axon/libaxon_pjrt.so")
    except Exception:
        hook = None
    mod = types.ModuleType("antenv.axon_hooks")
    mod.get_axon_ntff_profile_hook = lambda: hook
    mod.set_axon_ntff_profile_hook = lambda h: None
    import antenv
    sys.modules["antenv.axon_hooks"] = mod
    antenv.axon_hooks = mod


def kernel(**inputs):
    (mem_new, spike_new), _ = run(inputs, trace=False)
    return mem_new, spike_new


# revision 18
# speedup vs baseline: 1.0631x; 1.0189x over previous
"""Trainium2 Bass kernel for the DendriticLayer LIF problem.

Math (reference):
    mask[r, c] = (r % 4) == (c // 1024)            # block-diagonal per branch
    dense      = (x @ (W*mask).T + b).reshape(B, OUT, 4)
    d_new      = beta * d_input + (1-beta) * dense
    l_input    = d_new.sum(-1)
    mem_new    = alpha*mem + (1-alpha)*l_input - spike
    spike_new  = (mem_new - 1 > 0)

Because the mask is block-diagonal, row o*4+j of W only touches input block j.
Folding the per-row scales (1-alpha[o])*(1-beta[o,j]) into those blocks and
concatenating the 4 blocks along the contraction axis turns everything into a
single dense matmul:

    V[j*1024+k, o] = (1-alpha[o]) * (1-beta[o,j]) * W[o*4+j, j*1024+k]
    c2[o]          = (1-alpha[o]) * sum_j (1-beta[o,j]) * b[o*4+j]
    mem_new        = alpha*mem - spike + c2 + x @ V          (+ beta*d_input
                                                              term, host-side,
                                                              zero by spec)

On device (per core): 8 cores = 2 batch halves x 4 output quarters, so each
core computes a [512 out, 512 batch] block of (x @ V).T:

    psum[o, b] = sum_{k-pairs} Vq[2k, o]*x[2k, b] + Vq[2k+1, o]*x[2k+1, b]

using fp8e4m3 DoubleRow matmuls (2 fp8 weights per PE cell -> 256-deep
contraction per instruction; measured 216 ns per N=512 MM, i.e. 2x bf16
throughput).  x is 0/1 so it is exact in fp8; V is scaled by 2^11 so the
psum sits in e4m3 range and the PSUM->SBUF evacuation is a plain fp8 cast
(output = 256 KB).  Measured end-to-end rel err ~6e-4 vs the 2e-2 gate.

DMA: the host packs x|V chunk-major so every chunk is one fully contiguous
HBM span (best DMA efficiency), and chunks alternate between the two HWDGE
rings (Scalar engine's qActDynamicHW and Sync's qSPDynamicHW) so triggers
and transfers overlap.  Scalar goes first because its preamble retires
~1.3us before Sync's.  A few warm-up matmuls on an (uninitialized) tile
keep the PE busy from the moment the framework barrier drops so HAM
un-throttles the clock as early as possible.  The last chunk's matmuls are
grouped per PSUM bank so each bank's evacuation + store DMA overlaps the
remaining banks' matmuls.  The LIF elementwise update is a cheap host-side
epilogue.
"""

import os
import sys

import numpy as np
import ml_dtypes

for _p in ("/opt/trn_rl_repo",):
    if os.path.isdir(_p) and _p not in sys.path:
        sys.path.append(_p)

import concourse.bass as bass  # noqa: E402
import concourse.tile as tile  # noqa: E402
from concourse import bacc, mybir  # noqa: E402
from concourse._compat import with_exitstack  # noqa: E402
from concourse import bass_utils  # noqa: E402

# Problem shapes (hardcoded per harness contract)
B, IN, OUT, NB = 1024, 4096, 2048, 4
NCORES = 8
BH = B // 2                # 512 batch rows per core (2 halves)
OQ = OUT // 4              # 512 output rows per core (4 quarters)
P = 128                    # partition dim
KT = IN // P               # 32 contraction tiles of 128
KP = KT // 2               # 16 DoubleRow k-pairs
OTILES = OQ // P           # 4 output partition tiles
CW = BH + OQ               # stream columns per k-tile (x | v)
VTH = 1.0
NWARM = 34                 # warm-up matmuls: bridge preamble -> first data
SC = float(2 ** 11)        # V scale: psum absmax ~164 < e4m3 max 240

# k-tiles per DMA chunk (even so DoubleRow pairs stay inside a chunk) and
# the HWDGE ring each loads on (True = sync, False = scalar).  Uniform
# small chunks alternating rings: the ring loads self-balance across the
# whole timeline, so one slow ring cannot starve the PE for long.
CHUNKS = [2] * 16
RINGS = [1, 0] * 8
assert sum(CHUNKS) == KT and all(c % 2 == 0 for c in CHUNKS)

BF16 = mybir.dt.bfloat16
FP8 = mybir.dt.float8e4
F32 = mybir.dt.float32
BF16_NP = ml_dtypes.bfloat16
FP8_NP = ml_dtypes.float8_e4m3
DR = mybir.MatmulPerfMode.DoubleRow


@with_exitstack
def _body(ctx, tc, outt, sv):
    nc = tc.nc

    svpool = ctx.enter_context(tc.tile_pool(name="svpool", bufs=1))
    opool = ctx.enter_context(tc.tile_pool(name="opool", bufs=1))
    wpool = ctx.enter_context(tc.tile_pool(name="wpool", bufs=1))
    ppool = ctx.enter_context(tc.tile_pool(name="ppool", bufs=1, space="PSUM"))

    # PE warm-up: matmuls on a zeroed tile (psum never read), dependent only
    # on a VectorE memset, so they start right after the framework preamble
    # barrier drops -- HAM's 3.4us activity window starts counting as early
    # as possible.
    zt = wpool.tile([P, P], BF16, name="zt")
    nc.vector.memset(zt[:], 0.0)
    ps_warm = ppool.tile([P, P], F32, name="ps_warm")
    for w in range(NWARM):
        nc.tensor.matmul(ps_warm[:], zt[:], zt[:], start=True, stop=True,
                         skip_group_check=True)

    # Streaming loads: each chunk is one fully contiguous HBM span
    # (host packs chunk-major), alternating scalar/sync HWDGE rings.
    sg, kstart = [], []
    k0 = 0
    for g, ck in enumerate(CHUNKS):
        kstart.append(k0)
        t_ = svpool.tile([P, ck, CW], FP8, name=f"sg{g}")
        src = sv[k0 * P * CW:(k0 + ck) * P * CW].rearrange(
            "(p k c) -> p k c", p=P, c=CW)
        eng = nc.sync if RINGS[g] else nc.scalar
        eng.dma_start(t_[:], src)
        sg.append(t_)
        k0 += ck

    ps = [ppool.tile([P, BH], F32, name=f"ps{t}") for t in range(OTILES)]
    # One shared staging tile so the store DMAs cover multiple banks with
    # wider (1.5KB) per-partition descriptors; its writers are the 4 copies
    # only, so no false cross-engine deps arise.
    out_s = opool.tile([P, OTILES * BH], FP8, name="out_s")

    # Dense accumulation: 16 DoubleRow k-pairs x 4 o-tiles.  The last chunk
    # runs bank-major so each bank finishes (and drains) while later banks
    # still compute.
    last = len(CHUNKS) - 1
    for g, ck in enumerate(CHUNKS):
        pairs = [(kk, (kstart[g] + kk) // 2) for kk in range(0, ck, 2)]
        order = ([(t, kk, kp) for t in range(OTILES) for kk, kp in pairs]
                 if g == last else
                 [(t, kk, kp) for kk, kp in pairs for t in range(OTILES)])
        for t, kk, kp in order:
            x_ap = sg[g][:, kk:kk + 2, 0:BH]
            v_ap = sg[g][:, kk:kk + 2, BH + t * P:BH + (t + 1) * P]
            nc.tensor.matmul(
                ps[t][:], v_ap, x_ap,
                start=(kp == 0), stop=(kp == KP - 1),
                perf_mode=DR,
            )
            if g == last and kp == KP - 1:
                # Evacuate this bank (f32 psum -> fp8 cast), overlapping the
                # remaining banks' matmuls.  ScalarE takes banks 0/2, VectorE
                # banks 1/3 so the engines drain in parallel.  Banks 0-2 go
                # out in one wide DMA as soon as they are down; bank 3 (the
                # critical path) goes alone on the other ring.
                sl = slice(t * BH, (t + 1) * BH)
                if t == 3:
                    # Critical-path bank: split the cast across both engines.
                    h = BH // 2
                    nc.scalar.copy(out_s[:, t * BH:t * BH + h],
                                   ps[t][:, 0:h])
                    nc.vector.tensor_copy(out_s[:, t * BH + h:(t + 1) * BH],
                                          ps[t][:, h:BH])
                elif t % 2 == 0:
                    nc.scalar.copy(out_s[:, sl], ps[t][:])
                else:
                    nc.vector.tensor_copy(out_s[:, sl], ps[t][:])
                if t == 2:
                    nc.sync.dma_start(outt[:, 0:3 * BH], out_s[:, 0:3 * BH])
                elif t == 3:
                    nc.scalar.dma_start(outt[:, 3 * BH:], out_s[:, sl])


_CACHE = {}


def build():
    if "nc" in _CACHE:
        return _CACHE["nc"]
    nc = bacc.Bacc(
        "TRN2",
        target_bir_lowering=False,
        debug=False,
        enable_asserts=False,
        num_devices=NCORES,
    )
    sv = nc.dram_tensor("sv", [KT * P * CW], FP8, kind="ExternalInput").ap()
    outt = nc.dram_tensor("outt", [P, OTILES * BH], FP8,
                          kind="ExternalOutput").ap()
    with tile.TileContext(nc) as tc:
        _body(tc, outt, sv)
    nc.compile()
    _CACHE["nc"] = nc
    return nc


def _sigmoid64(x):
    return 1.0 / (1.0 + np.exp(-x.astype(np.float64)))


def prep_host(inputs):
    """Fold scales into weights; build per-core input maps."""
    W = np.asarray(inputs["W"])
    b = np.asarray(inputs["b"])
    alpha = _sigmoid64(np.asarray(inputs["tau_m"]))        # [OUT]
    beta = _sigmoid64(np.asarray(inputs["tau_n"]))         # [OUT, NB]
    S = IN // NB

    W4 = W.reshape(OUT, NB, IN)                            # row o*4+j = W4[o, j]
    s = (1.0 - alpha)[:, None] * (1.0 - beta)              # [OUT, NB] f64
    blocks = [
        (W4[:, j, j * S:(j + 1) * S].astype(np.float64) * s[:, j:j + 1]).T
        for j in range(NB)
    ]
    V = np.concatenate(blocks, axis=0)                     # [IN, OUT] f64
    c2 = ((1.0 - alpha) * np.sum((1.0 - beta) * b.reshape(OUT, NB).astype(np.float64), axis=1))

    # X packed partition-major per k-tile, split into batch halves:
    # xk[h][k, p, b] = X[h*BH + b, k*128+p], exact in fp8 (x is 0/1)
    Xt = np.asarray(inputs["input_spike"]).T               # [IN, B]
    xk = [
        Xt[:, h * BH:(h + 1) * BH].astype(FP8_NP).reshape(KT, P, BH)
        for h in range(2)
    ]
    # V quarters, scaled into e4m3 range
    Vs = np.clip(V * SC, -240.0, 240.0).astype(FP8_NP)     # [IN, OUT]

    in_maps = []
    for c in range(NCORES):
        h, q = c % 2, c // 2
        vkq = Vs[:, q * OQ:(q + 1) * OQ].reshape(KT, P, OQ)
        stream = np.concatenate([xk[h], vkq], axis=2)      # [KT, P, CW]
        # chunk-major: each chunk g is a contiguous [P, ck*CW] block
        parts, k0 = [], 0
        for ck in CHUNKS:
            parts.append(np.ascontiguousarray(
                stream[k0:k0 + ck].transpose(1, 0, 2)).reshape(-1))
            k0 += ck
        SV = np.concatenate(parts)
        in_maps.append({"sv": SV})
    return in_maps, alpha, beta, c2


def finish_host(shards, inputs, alpha, beta, c2):
    l_part = np.empty((B, OUT), dtype=np.float32)
    inv_sc = np.float32(1.0 / SC)
    for c in range(NCORES):
        h, q = c % 2, c // 2
        arr = shards[c].astype(np.float32)                 # [P, OTILES*BH]
        Oc = arr.reshape(P, OTILES, BH).transpose(1, 0, 2).reshape(OQ, BH)
        l_part[h * BH:(h + 1) * BH, q * OQ:(q + 1) * OQ] = Oc.T * inv_sc
    a32 = alpha.astype(np.float32)[None, :]
    c32 = c2.astype(np.float32)[None, :]
    mem = np.asarray(inputs["mem"])
    spk = np.asarray(inputs["spike"])
    mem_new = mem * a32 - spk + c32 + l_part               # fp32 elementwise
    d_input = np.asarray(inputs["d_input"])
    if d_input.any():
        corr = (
            np.einsum("boj,oj->bo", d_input.astype(np.float64), beta)
            * (1.0 - alpha)[None, :]
        ).astype(np.float32)
        mem_new = mem_new + corr
    spike_new = ((mem_new - np.float32(VTH)) > 0).astype(np.float32)
    return mem_new, spike_new


def _axon_reset():
    """Recover wedged NeuronCores (NRT_EXEC_UNIT_UNRECOVERABLE) via the
    axon client's reset entry point."""
    try:
        import ctypes
        import jax
        jax.devices()
        lib = ctypes.CDLL("/opt/axon/libaxon_pjrt.so")
        lib.axon_reset.restype = ctypes.c_int64
        lib.axon_reset()
    except Exception:
        pass


def run(inputs, trace=False):
    nc = build()
    in_maps, alpha, beta, c2 = prep_host(inputs)
    kwargs = {}
    if trace:
        bass_utils.upload_artifacts = lambda tmpdir: tmpdir
        _ensure_ntff_hook()
        kwargs["trace"] = True
    try:
        res = bass_utils.run_bass_kernel_spmd(
            nc, in_maps, core_ids=list(range(NCORES)), **kwargs
        )
    except Exception:
        _axon_reset()
        res = bass_utils.run_bass_kernel_spmd(
            nc, in_maps, core_ids=list(range(NCORES)), **kwargs
        )
    shards = [res.results[c]["outt"] for c in range(NCORES)]
    mem_new, spike_new = finish_host(shards, inputs, alpha, beta, c2)
    return (mem_new, spike_new), res


def _ensure_ntff_hook():
    try:
        from antenv.axon_hooks import get_axon_ntff_profile_hook  # noqa: F401
        return
    except ImportError:
        pass
    import types
    try:
        import trn_agent_boot.trn_boot as tb
        hook = tb._ntff_profile_via_ctypes("/opt/axon/libaxon_pjrt.so")
    except Exception:
        hook = None
    mod = types.ModuleType("antenv.axon_hooks")
    mod.get_axon_ntff_profile_hook = lambda: hook
    mod.set_axon_ntff_profile_hook = lambda h: None
    import antenv
    sys.modules["antenv.axon_hooks"] = mod
    antenv.axon_hooks = mod


def kernel(**inputs):
    (mem_new, spike_new), _ = run(inputs, trace=False)
    return mem_new, spike_new


# revision 19
# speedup vs baseline: 1.0651x; 1.0019x over previous
"""Trainium2 Bass kernel for the DendriticLayer LIF problem.

Math (reference):
    mask[r, c] = (r % 4) == (c // 1024)            # block-diagonal per branch
    dense      = (x @ (W*mask).T + b).reshape(B, OUT, 4)
    d_new      = beta * d_input + (1-beta) * dense
    l_input    = d_new.sum(-1)
    mem_new    = alpha*mem + (1-alpha)*l_input - spike
    spike_new  = (mem_new - 1 > 0)

Because the mask is block-diagonal, row o*4+j of W only touches input block j.
Folding the per-row scales (1-alpha[o])*(1-beta[o,j]) into those blocks and
concatenating the 4 blocks along the contraction axis turns everything into a
single dense matmul:

    V[j*1024+k, o] = (1-alpha[o]) * (1-beta[o,j]) * W[o*4+j, j*1024+k]
    c2[o]          = (1-alpha[o]) * sum_j (1-beta[o,j]) * b[o*4+j]
    mem_new        = alpha*mem - spike + c2 + x @ V          (+ beta*d_input
                                                              term, host-side,
                                                              zero by spec)

On device (per core): 8 cores = 2 batch halves x 4 output quarters, so each
core computes a [512 out, 512 batch] block of (x @ V).T:

    psum[o, b] = sum_{k-pairs} Vq[2k, o]*x[2k, b] + Vq[2k+1, o]*x[2k+1, b]

using fp8e4m3 DoubleRow matmuls (2 fp8 weights per PE cell -> 256-deep
contraction per instruction; measured 216 ns per N=512 MM, i.e. 2x bf16
throughput).  x is 0/1 so it is exact in fp8; V is scaled by 2^11 so the
psum sits in e4m3 range and the PSUM->SBUF evacuation is a plain fp8 cast
(output = 256 KB).  Measured end-to-end rel err ~6e-4 vs the 2e-2 gate.

DMA: the host packs x|V chunk-major so every chunk is one fully contiguous
HBM span (best DMA efficiency), and chunks alternate between the two HWDGE
rings (Scalar engine's qActDynamicHW and Sync's qSPDynamicHW) so triggers
and transfers overlap.  Scalar goes first because its preamble retires
~1.3us before Sync's.  A few warm-up matmuls on an (uninitialized) tile
keep the PE busy from the moment the framework barrier drops so HAM
un-throttles the clock as early as possible.  The last chunk's matmuls are
grouped per PSUM bank so each bank's evacuation + store DMA overlaps the
remaining banks' matmuls.  The LIF elementwise update is a cheap host-side
epilogue.
"""

import os
import sys

import numpy as np
import ml_dtypes

for _p in ("/opt/trn_rl_repo",):
    if os.path.isdir(_p) and _p not in sys.path:
        sys.path.append(_p)

import concourse.bass as bass  # noqa: E402
import concourse.tile as tile  # noqa: E402
from concourse import bacc, mybir  # noqa: E402
from concourse._compat import with_exitstack  # noqa: E402
from concourse import bass_utils  # noqa: E402

# Problem shapes (hardcoded per harness contract)
B, IN, OUT, NB = 1024, 4096, 2048, 4
NCORES = 8
BH = B // 2                # 512 batch rows per core (2 halves)
OQ = OUT // 4              # 512 output rows per core (4 quarters)
P = 128                    # partition dim
KT = IN // P               # 32 contraction tiles of 128
KP = KT // 2               # 16 DoubleRow k-pairs
OTILES = OQ // P           # 4 output partition tiles
CW = BH + OQ               # stream columns per k-tile (x | v)
VTH = 1.0
NWARM = 34                 # warm-up matmuls: bridge preamble -> first data
SC = float(2 ** 11)        # V scale: psum absmax ~164 < e4m3 max 240

# k-tiles per DMA chunk (even so DoubleRow pairs stay inside a chunk) and
# the HWDGE ring each loads on (True = sync, False = scalar).  Uniform
# small chunks alternating rings: the ring loads self-balance across the
# whole timeline, so one slow ring cannot starve the PE for long.
CHUNKS = [2] * 16
RINGS = [1, 0] * 8
assert sum(CHUNKS) == KT and all(c % 2 == 0 for c in CHUNKS)

BF16 = mybir.dt.bfloat16
FP8 = mybir.dt.float8e4
F32 = mybir.dt.float32
BF16_NP = ml_dtypes.bfloat16
FP8_NP = ml_dtypes.float8_e4m3
DR = mybir.MatmulPerfMode.DoubleRow


@with_exitstack
def _body(ctx, tc, outt, sv):
    nc = tc.nc

    svpool = ctx.enter_context(tc.tile_pool(name="svpool", bufs=1))
    opool = ctx.enter_context(tc.tile_pool(name="opool", bufs=1))
    wpool = ctx.enter_context(tc.tile_pool(name="wpool", bufs=1))
    ppool = ctx.enter_context(tc.tile_pool(name="ppool", bufs=1, space="PSUM"))

    # PE warm-up: matmuls on a zeroed tile (psum never read), dependent only
    # on a VectorE memset, so they start right after the framework preamble
    # barrier drops -- HAM's 3.4us activity window starts counting as early
    # as possible.
    zt = wpool.tile([P, P], BF16, name="zt")
    nc.vector.memset(zt[:], 0.0)
    ps_warm = ppool.tile([P, P], F32, name="ps_warm")
    for w in range(NWARM):
        nc.tensor.matmul(ps_warm[:], zt[:], zt[:], start=True, stop=True,
                         skip_group_check=True)

    # Streaming loads: each chunk is one fully contiguous HBM span
    # (host packs chunk-major), alternating scalar/sync HWDGE rings.
    sg, kstart = [], []
    k0 = 0
    for g, ck in enumerate(CHUNKS):
        kstart.append(k0)
        t_ = svpool.tile([P, ck, CW], FP8, name=f"sg{g}")
        src = sv[k0 * P * CW:(k0 + ck) * P * CW].rearrange(
            "(p k c) -> p k c", p=P, c=CW)
        eng = nc.sync if RINGS[g] else nc.scalar
        eng.dma_start(t_[:], src)
        sg.append(t_)
        k0 += ck

    ps = [ppool.tile([P, BH], F32, name=f"ps{t}") for t in range(OTILES)]
    # One shared staging tile so the store DMAs cover multiple banks with
    # wider (1.5KB) per-partition descriptors; its writers are the 4 copies
    # only, so no false cross-engine deps arise.
    out_s = opool.tile([P, OTILES * BH], FP8, name="out_s")

    # Dense accumulation: 16 DoubleRow k-pairs x 4 o-tiles.  The last chunk
    # runs bank-major so each bank finishes (and drains) while later banks
    # still compute.
    last = len(CHUNKS) - 1
    for g, ck in enumerate(CHUNKS):
        pairs = [(kk, (kstart[g] + kk) // 2) for kk in range(0, ck, 2)]
        order = ([(t, kk, kp) for t in range(OTILES) for kk, kp in pairs]
                 if g == last else
                 [(t, kk, kp) for kk, kp in pairs for t in range(OTILES)])
        for t, kk, kp in order:
            x_ap = sg[g][:, kk:kk + 2, 0:BH]
            v_ap = sg[g][:, kk:kk + 2, BH + t * P:BH + (t + 1) * P]
            nc.tensor.matmul(
                ps[t][:], v_ap, x_ap,
                start=(kp == 0), stop=(kp == KP - 1),
                perf_mode=DR,
            )
            if g == last and kp == KP - 1:
                # Evacuate this bank (f32 psum -> fp8 cast), overlapping the
                # remaining banks' matmuls.  ScalarE takes banks 0/2, VectorE
                # banks 1/3 so the engines drain in parallel.  Banks 0-2 go
                # out in one wide DMA as soon as they are down; bank 3 (the
                # critical path) goes alone on the other ring.
                # ScalarE casts banks 0/2, VectorE banks 1/3 (parallel
                # engines).  Banks 0-2 leave in one wide sync-ring DMA; the
                # critical bank 3 leaves on the scalar ring, whose wait on
                # VectorE's cast is the last thing in the scalar queue (a
                # wait placed before a copy head-of-line blocks the engine).
                sl = slice(t * BH, (t + 1) * BH)
                if t % 2 == 0:
                    nc.scalar.copy(out_s[:, sl], ps[t][:])
                else:
                    nc.vector.tensor_copy(out_s[:, sl], ps[t][:])
                if t == 2:
                    nc.sync.dma_start(outt[:, 0:3 * BH], out_s[:, 0:3 * BH])
                elif t == 3:
                    nc.scalar.dma_start(outt[:, 3 * BH:], out_s[:, sl])


_CACHE = {}


def build():
    if "nc" in _CACHE:
        return _CACHE["nc"]
    nc = bacc.Bacc(
        "TRN2",
        target_bir_lowering=False,
        debug=False,
        enable_asserts=False,
        num_devices=NCORES,
    )
    sv = nc.dram_tensor("sv", [KT * P * CW], FP8, kind="ExternalInput").ap()
    outt = nc.dram_tensor("outt", [P, OTILES * BH], FP8,
                          kind="ExternalOutput").ap()
    with tile.TileContext(nc) as tc:
        _body(tc, outt, sv)
    nc.compile()
    _CACHE["nc"] = nc
    return nc


def _sigmoid64(x):
    return 1.0 / (1.0 + np.exp(-x.astype(np.float64)))


def prep_host(inputs):
    """Fold scales into weights; build per-core input maps."""
    W = np.asarray(inputs["W"])
    b = np.asarray(inputs["b"])
    alpha = _sigmoid64(np.asarray(inputs["tau_m"]))        # [OUT]
    beta = _sigmoid64(np.asarray(inputs["tau_n"]))         # [OUT, NB]
    S = IN // NB

    W4 = W.reshape(OUT, NB, IN)                            # row o*4+j = W4[o, j]
    s = (1.0 - alpha)[:, None] * (1.0 - beta)              # [OUT, NB] f64
    blocks = [
        (W4[:, j, j * S:(j + 1) * S].astype(np.float64) * s[:, j:j + 1]).T
        for j in range(NB)
    ]
    V = np.concatenate(blocks, axis=0)                     # [IN, OUT] f64
    c2 = ((1.0 - alpha) * np.sum((1.0 - beta) * b.reshape(OUT, NB).astype(np.float64), axis=1))

    # X packed partition-major per k-tile, split into batch halves:
    # xk[h][k, p, b] = X[h*BH + b, k*128+p], exact in fp8 (x is 0/1)
    Xt = np.asarray(inputs["input_spike"]).T               # [IN, B]
    xk = [
        Xt[:, h * BH:(h + 1) * BH].astype(FP8_NP).reshape(KT, P, BH)
        for h in range(2)
    ]
    # V quarters, scaled into e4m3 range
    Vs = np.clip(V * SC, -240.0, 240.0).astype(FP8_NP)     # [IN, OUT]

    in_maps = []
    for c in range(NCORES):
        h, q = c % 2, c // 2
        vkq = Vs[:, q * OQ:(q + 1) * OQ].reshape(KT, P, OQ)
        stream = np.concatenate([xk[h], vkq], axis=2)      # [KT, P, CW]
        # chunk-major: each chunk g is a contiguous [P, ck*CW] block
        parts, k0 = [], 0
        for ck in CHUNKS:
            parts.append(np.ascontiguousarray(
                stream[k0:k0 + ck].transpose(1, 0, 2)).reshape(-1))
            k0 += ck
        SV = np.concatenate(parts)
        in_maps.append({"sv": SV})
    return in_maps, alpha, beta, c2


def finish_host(shards, inputs, alpha, beta, c2):
    l_part = np.empty((B, OUT), dtype=np.float32)
    inv_sc = np.float32(1.0 / SC)
    for c in range(NCORES):
        h, q = c % 2, c // 2
        arr = shards[c].astype(np.float32)                 # [P, OTILES*BH]
        Oc = arr.reshape(P, OTILES, BH).transpose(1, 0, 2).reshape(OQ, BH)
        l_part[h * BH:(h + 1) * BH, q * OQ:(q + 1) * OQ] = Oc.T * inv_sc
    a32 = alpha.astype(np.float32)[None, :]
    c32 = c2.astype(np.float32)[None, :]
    mem = np.asarray(inputs["mem"])
    spk = np.asarray(inputs["spike"])
    mem_new = mem * a32 - spk + c32 + l_part               # fp32 elementwise
    d_input = np.asarray(inputs["d_input"])
    if d_input.any():
        corr = (
            np.einsum("boj,oj->bo", d_input.astype(np.float64), beta)
            * (1.0 - alpha)[None, :]
        ).astype(np.float32)
        mem_new = mem_new + corr
    spike_new = ((mem_new - np.float32(VTH)) > 0).astype(np.float32)
    return mem_new, spike_new


def _axon_reset():
    """Recover wedged NeuronCores (NRT_EXEC_UNIT_UNRECOVERABLE) via the
    axon client's reset entry point."""
    try:
        import ctypes
        import jax
        jax.devices()
        lib = ctypes.CDLL("/opt/axon/libaxon_pjrt.so")
        lib.axon_reset.restype = ctypes.c_int64
        lib.axon_reset()
    except Exception:
        pass


def run(inputs, trace=False):
    nc = build()
    in_maps, alpha, beta, c2 = prep_host(inputs)
    kwargs = {}
    if trace:
        bass_utils.upload_artifacts = lambda tmpdir: tmpdir
        _ensure_ntff_hook()
        kwargs["trace"] = True
    try:
        res = bass_utils.run_bass_kernel_spmd(
            nc, in_maps, core_ids=list(range(NCORES)), **kwargs
        )
    except Exception:
        _axon_reset()
        res = bass_utils.run_bass_kernel_spmd(
            nc, in_maps, core_ids=list(range(NCORES)), **kwargs
        )
    shards = [res.results[c]["outt"] for c in range(NCORES)]
    mem_new, spike_new = finish_host(shards, inputs, alpha, beta, c2)
    return (mem_new, spike_new), res


def _ensure_ntff_hook():
    try:
        from antenv.axon_hooks import get_axon_ntff_profile_hook  # noqa: F401
        return
    except ImportError:
        pass
    import types
    try:
        import trn_agent_boot.trn_boot as tb
        hook = tb._ntff_profile_via_ctypes("/opt/axon/libaxon_pjrt.so")
    except Exception:
        hook = None
    mod = types.ModuleType("antenv.axon_hooks")
    mod.get_axon_ntff_profile_hook = lambda: hook
    mod.set_axon_ntff_profile_hook = lambda h: None
    import antenv
    sys.modules["antenv.axon_hooks"] = mod
    antenv.axon_hooks = mod


def kernel(**inputs):
    (mem_new, spike_new), _ = run(inputs, trace=False)
    return mem_new, spike_new


# revision 20
# speedup vs baseline: 1.0680x; 1.0027x over previous
"""Trainium2 Bass kernel for the DendriticLayer LIF problem.

Math (reference):
    mask[r, c] = (r % 4) == (c // 1024)            # block-diagonal per branch
    dense      = (x @ (W*mask).T + b).reshape(B, OUT, 4)
    d_new      = beta * d_input + (1-beta) * dense
    l_input    = d_new.sum(-1)
    mem_new    = alpha*mem + (1-alpha)*l_input - spike
    spike_new  = (mem_new - 1 > 0)

Because the mask is block-diagonal, row o*4+j of W only touches input block j.
Folding the per-row scales (1-alpha[o])*(1-beta[o,j]) into those blocks and
concatenating the 4 blocks along the contraction axis turns everything into a
single dense matmul:

    V[j*1024+k, o] = (1-alpha[o]) * (1-beta[o,j]) * W[o*4+j, j*1024+k]
    c2[o]          = (1-alpha[o]) * sum_j (1-beta[o,j]) * b[o*4+j]
    mem_new        = alpha*mem - spike + c2 + x @ V          (+ beta*d_input
                                                              term, host-side,
                                                              zero by spec)

On device (per core): 8 cores = 2 batch halves x 4 output quarters, so each
core computes a [512 out, 512 batch] block of (x @ V).T:

    psum[o, b] = sum_{k-pairs} Vq[2k, o]*x[2k, b] + Vq[2k+1, o]*x[2k+1, b]

using fp8e4m3 DoubleRow matmuls (2 fp8 weights per PE cell -> 256-deep
contraction per instruction; measured 216 ns per N=512 MM, i.e. 2x bf16
throughput).  x is 0/1 so it is exact in fp8; V is scaled by 2^11 so the
psum sits in e4m3 range and the PSUM->SBUF evacuation is a plain fp8 cast
(output = 256 KB).  Measured end-to-end rel err ~6e-4 vs the 2e-2 gate.

DMA: the host packs x|V chunk-major so every chunk is one fully contiguous
HBM span (best DMA efficiency), and chunks alternate between the two HWDGE
rings (Scalar engine's qActDynamicHW and Sync's qSPDynamicHW) so triggers
and transfers overlap.  Scalar goes first because its preamble retires
~1.3us before Sync's.  A few warm-up matmuls on an (uninitialized) tile
keep the PE busy from the moment the framework barrier drops so HAM
un-throttles the clock as early as possible.  The last chunk's matmuls are
grouped per PSUM bank so each bank's evacuation + store DMA overlaps the
remaining banks' matmuls.  The LIF elementwise update is a cheap host-side
epilogue.
"""

import os
import sys

import numpy as np
import ml_dtypes

for _p in ("/opt/trn_rl_repo",):
    if os.path.isdir(_p) and _p not in sys.path:
        sys.path.append(_p)

import concourse.bass as bass  # noqa: E402
import concourse.tile as tile  # noqa: E402
from concourse import bacc, mybir  # noqa: E402
from concourse._compat import with_exitstack  # noqa: E402
from concourse import bass_utils  # noqa: E402

# Problem shapes (hardcoded per harness contract)
B, IN, OUT, NB = 1024, 4096, 2048, 4
NCORES = 8
BH = B // 2                # 512 batch rows per core (2 halves)
OQ = OUT // 4              # 512 output rows per core (4 quarters)
P = 128                    # partition dim
KT = IN // P               # 32 contraction tiles of 128
KP = KT // 2               # 16 DoubleRow k-pairs
OTILES = OQ // P           # 4 output partition tiles
CW = BH + OQ               # stream columns per k-tile (x | v)
VTH = 1.0
NWARM = 34                 # warm-up matmuls: bridge preamble -> first data
SC = float(2 ** 11)        # V scale: psum absmax ~164 < e4m3 max 240

# k-tiles per DMA chunk (even so DoubleRow pairs stay inside a chunk) and
# the HWDGE ring each loads on (True = sync, False = scalar).  Small chunks
# alternating rings early (fine-grained PE chasing while the rings ramp),
# larger late (fewer completion semaphores to settle at close).
CHUNKS = [2, 2, 2, 2, 2, 2, 4, 4, 4, 4, 4]
RINGS = [1, 0, 1, 0, 1, 0, 1, 0, 1, 0, 1]
assert sum(CHUNKS) == KT and all(c % 2 == 0 for c in CHUNKS)

BF16 = mybir.dt.bfloat16
FP8 = mybir.dt.float8e4
F32 = mybir.dt.float32
BF16_NP = ml_dtypes.bfloat16
FP8_NP = ml_dtypes.float8_e4m3
DR = mybir.MatmulPerfMode.DoubleRow


@with_exitstack
def _body(ctx, tc, outt, sv):
    nc = tc.nc

    svpool = ctx.enter_context(tc.tile_pool(name="svpool", bufs=1))
    opool = ctx.enter_context(tc.tile_pool(name="opool", bufs=1))
    wpool = ctx.enter_context(tc.tile_pool(name="wpool", bufs=1))
    ppool = ctx.enter_context(tc.tile_pool(name="ppool", bufs=1, space="PSUM"))

    # PE warm-up: matmuls on a zeroed tile (psum never read), dependent only
    # on a VectorE memset, so they start right after the framework preamble
    # barrier drops -- HAM's 3.4us activity window starts counting as early
    # as possible.
    zt = wpool.tile([P, P], BF16, name="zt")
    nc.vector.memset(zt[:], 0.0)
    ps_warm = ppool.tile([P, P], F32, name="ps_warm")
    for w in range(NWARM):
        nc.tensor.matmul(ps_warm[:], zt[:], zt[:], start=True, stop=True,
                         skip_group_check=True)

    # Streaming loads: each chunk is one fully contiguous HBM span
    # (host packs chunk-major), alternating scalar/sync HWDGE rings.
    sg, kstart = [], []
    k0 = 0
    for g, ck in enumerate(CHUNKS):
        kstart.append(k0)
        t_ = svpool.tile([P, ck, CW], FP8, name=f"sg{g}")
        src = sv[k0 * P * CW:(k0 + ck) * P * CW].rearrange(
            "(p k c) -> p k c", p=P, c=CW)
        eng = nc.sync if RINGS[g] else nc.scalar
        eng.dma_start(t_[:], src)
        sg.append(t_)
        k0 += ck

    ps = [ppool.tile([P, BH], F32, name=f"ps{t}") for t in range(OTILES)]
    # One shared staging tile so the store DMAs cover multiple banks with
    # wider (1.5KB) per-partition descriptors; its writers are the 4 copies
    # only, so no false cross-engine deps arise.
    out_s = opool.tile([P, OTILES * BH], FP8, name="out_s")

    # Dense accumulation: 16 DoubleRow k-pairs x 4 o-tiles.  The last chunk
    # runs bank-major so each bank finishes (and drains) while later banks
    # still compute.
    last = len(CHUNKS) - 1
    for g, ck in enumerate(CHUNKS):
        pairs = [(kk, (kstart[g] + kk) // 2) for kk in range(0, ck, 2)]
        order = ([(t, kk, kp) for t in range(OTILES) for kk, kp in pairs]
                 if g == last else
                 [(t, kk, kp) for kk, kp in pairs for t in range(OTILES)])
        for t, kk, kp in order:
            x_ap = sg[g][:, kk:kk + 2, 0:BH]
            v_ap = sg[g][:, kk:kk + 2, BH + t * P:BH + (t + 1) * P]
            nc.tensor.matmul(
                ps[t][:], v_ap, x_ap,
                start=(kp == 0), stop=(kp == KP - 1),
                perf_mode=DR,
            )
            if g == last and kp == KP - 1:
                # Evacuate this bank (f32 psum -> fp8 cast), overlapping the
                # remaining banks' matmuls.  ScalarE takes banks 0/2, VectorE
                # banks 1/3 so the engines drain in parallel.  Banks 0-2 go
                # out in one wide DMA as soon as they are down; bank 3 (the
                # critical path) goes alone on the other ring.
                # ScalarE casts banks 0/2, VectorE banks 1/3 (parallel
                # engines).  Banks 0-2 leave in one wide sync-ring DMA; the
                # critical bank 3 leaves on the scalar ring, whose wait on
                # VectorE's cast is the last thing in the scalar queue (a
                # wait placed before a copy head-of-line blocks the engine).
                sl = slice(t * BH, (t + 1) * BH)
                if t % 2 == 0:
                    nc.scalar.copy(out_s[:, sl], ps[t][:])
                else:
                    nc.vector.tensor_copy(out_s[:, sl], ps[t][:])
                if t == 2:
                    nc.sync.dma_start(outt[:, 0:3 * BH], out_s[:, 0:3 * BH])
                elif t == 3:
                    nc.scalar.dma_start(outt[:, 3 * BH:], out_s[:, sl])


_CACHE = {}


def build():
    if "nc" in _CACHE:
        return _CACHE["nc"]
    nc = bacc.Bacc(
        "TRN2",
        target_bir_lowering=False,
        debug=False,
        enable_asserts=False,
        num_devices=NCORES,
    )
    sv = nc.dram_tensor("sv", [KT * P * CW], FP8, kind="ExternalInput").ap()
    outt = nc.dram_tensor("outt", [P, OTILES * BH], FP8,
                          kind="ExternalOutput").ap()
    with tile.TileContext(nc) as tc:
        _body(tc, outt, sv)
    nc.compile()
    _CACHE["nc"] = nc
    return nc


def _sigmoid64(x):
    return 1.0 / (1.0 + np.exp(-x.astype(np.float64)))


def prep_host(inputs):
    """Fold scales into weights; build per-core input maps."""
    W = np.asarray(inputs["W"])
    b = np.asarray(inputs["b"])
    alpha = _sigmoid64(np.asarray(inputs["tau_m"]))        # [OUT]
    beta = _sigmoid64(np.asarray(inputs["tau_n"]))         # [OUT, NB]
    S = IN // NB

    W4 = W.reshape(OUT, NB, IN)                            # row o*4+j = W4[o, j]
    s = (1.0 - alpha)[:, None] * (1.0 - beta)              # [OUT, NB] f64
    blocks = [
        (W4[:, j, j * S:(j + 1) * S].astype(np.float64) * s[:, j:j + 1]).T
        for j in range(NB)
    ]
    V = np.concatenate(blocks, axis=0)                     # [IN, OUT] f64
    c2 = ((1.0 - alpha) * np.sum((1.0 - beta) * b.reshape(OUT, NB).astype(np.float64), axis=1))

    # X packed partition-major per k-tile, split into batch halves:
    # xk[h][k, p, b] = X[h*BH + b, k*128+p], exact in fp8 (x is 0/1)
    Xt = np.asarray(inputs["input_spike"]).T               # [IN, B]
    xk = [
        Xt[:, h * BH:(h + 1) * BH].astype(FP8_NP).reshape(KT, P, BH)
        for h in range(2)
    ]
    # V quarters, scaled into e4m3 range
    Vs = np.clip(V * SC, -240.0, 240.0).astype(FP8_NP)     # [IN, OUT]

    in_maps = []
    for c in range(NCORES):
        h, q = c % 2, c // 2
        vkq = Vs[:, q * OQ:(q + 1) * OQ].reshape(KT, P, OQ)
        stream = np.concatenate([xk[h], vkq], axis=2)      # [KT, P, CW]
        # chunk-major: each chunk g is a contiguous [P, ck*CW] block
        parts, k0 = [], 0
        for ck in CHUNKS:
            parts.append(np.ascontiguousarray(
                stream[k0:k0 + ck].transpose(1, 0, 2)).reshape(-1))
            k0 += ck
        SV = np.concatenate(parts)
        in_maps.append({"sv": SV})
    return in_maps, alpha, beta, c2


def finish_host(shards, inputs, alpha, beta, c2):
    l_part = np.empty((B, OUT), dtype=np.float32)
    inv_sc = np.float32(1.0 / SC)
    for c in range(NCORES):
        h, q = c % 2, c // 2
        arr = shards[c].astype(np.float32)                 # [P, OTILES*BH]
        Oc = arr.reshape(P, OTILES, BH).transpose(1, 0, 2).reshape(OQ, BH)
        l_part[h * BH:(h + 1) * BH, q * OQ:(q + 1) * OQ] = Oc.T * inv_sc
    a32 = alpha.astype(np.float32)[None, :]
    c32 = c2.astype(np.float32)[None, :]
    mem = np.asarray(inputs["mem"])
    spk = np.asarray(inputs["spike"])
    mem_new = mem * a32 - spk + c32 + l_part               # fp32 elementwise
    d_input = np.asarray(inputs["d_input"])
    if d_input.any():
        corr = (
            np.einsum("boj,oj->bo", d_input.astype(np.float64), beta)
            * (1.0 - alpha)[None, :]
        ).astype(np.float32)
        mem_new = mem_new + corr
    spike_new = ((mem_new - np.float32(VTH)) > 0).astype(np.float32)
    return mem_new, spike_new


def _axon_reset():
    """Recover wedged NeuronCores (NRT_EXEC_UNIT_UNRECOVERABLE) via the
    axon client's reset entry point."""
    try:
        import ctypes
        import jax
        jax.devices()
        lib = ctypes.CDLL("/opt/axon/libaxon_pjrt.so")
        lib.axon_reset.restype = ctypes.c_int64
        lib.axon_reset()
    except Exception:
        pass


def run(inputs, trace=False):
    nc = build()
    in_maps, alpha, beta, c2 = prep_host(inputs)
    kwargs = {}
    if trace:
        bass_utils.upload_artifacts = lambda tmpdir: tmpdir
        _ensure_ntff_hook()
        kwargs["trace"] = True
    try:
        res = bass_utils.run_bass_kernel_spmd(
            nc, in_maps, core_ids=list(range(NCORES)), **kwargs
        )
    except Exception:
        _axon_reset()
        res = bass_utils.run_bass_kernel_spmd(
            nc, in_maps, core_ids=list(range(NCORES)), **kwargs
        )
    shards = [res.results[c]["outt"] for c in range(NCORES)]
    mem_new, spike_new = finish_host(shards, inputs, alpha, beta, c2)
    return (mem_new, spike_new), res


def _ensure_ntff_hook():
    try:
        from antenv.axon_hooks import get_axon_ntff_profile_hook  # noqa: F401
        return
    except ImportError:
        pass
    import types
    try:
        import trn_agent_boot.trn_boot as tb
        hook = tb._ntff_profile_via_ctypes("/opt/axon/libaxon_pjrt.so")
    except Exception:
        hook = None
    mod = types.ModuleType("antenv.axon_hooks")
    mod.get_axon_ntff_profile_hook = lambda: hook
    mod.set_axon_ntff_profile_hook = lambda h: None
    import antenv
    sys.modules["antenv.axon_hooks"] = mod
    antenv.axon_hooks = mod


def kernel(**inputs):
    (mem_new, spike_new), _ = run(inputs, trace=False)
    return mem_new, spike_new
